# revision 18
# baseline (speedup 1.0000x reference)
"""Dice loss kernel for Trainium2 (8 NeuronCores, SPMD data-parallel).

Problem: nn_DiceLoss — logits [8,19,512,512] f32, targets [8,512,512] int64.
  probs = softmax(logits, axis=1)
  PS[c] = sum_px probs[c,px]            (probs_sum)
  I[c]  = sum_px probs[t(px),px]*[t==c] (intersection)
  CT[c] = histogram(targets)            (counts; computed on host)
  dice  = (2I+1)/(PS+CT+1); loss = mean(1-dice)

Sharding: batch b -> core b. Each core reduces its [19, 512*512] slice to
per-class partials; host combines the 8 partial vectors and finishes.

Device layout (per core): classes-on-partitions, c-major rows (row = c*G+g):
  42 main tiles [114 = 19c x 6g, F=1024 pixels] + 1 remainder [76 = 19c x 4g, 1024].
  - ACT: E = exp(L) f32->bf16
  - PE:  S = blockdiag-ones.T @ E  -> per-pixel sum over the 19 classes,
         stacked 21 tiles deep in one PSUM tile [126, F]
  - DVE: r = approx 1/S (f32), cast bf16
  - PE:  broadcast r back to [114, F] (blockdiag.T @ r rows)
  - ACT: copy r-broadcast PSUM f32 -> SBUF bf16
  - DVE: tensor_tensor_reduce: W = E*r, accum_out = row sums -> PS partials
  - DMA: broadcast uint8 targets [6,F] -> [114,F] (19 small SBUF copies)
  - DVE: scalar_tensor_tensor: (T==c_row)*W, accum_out -> I partials
Outputs per core: out [4,128] f32 = PS/I partials for main + remainder rows.
"""

import functools
import sys

import numpy as np

sys.path.insert(0, "/opt/trn_rl_repo")

import ml_dtypes  # noqa: E402

B, C, H, W = 8, 19, 512, 512
HW = H * W  # 262144
F = 1024  # pixels per group column chunk
G_MAIN = 6  # groups per main tile -> 114 partitions
G_REM = 4  # groups in remainder tile -> 76 partitions
PXT = G_MAIN * F  # 6144 pixels per main tile
N_MAIN = 42  # main tiles (42*6144 = 258048)
REM_PX0 = N_MAIN * PXT  # 258048
BATCH = 21  # tiles per reciprocal batch ([126, F] PSUM stack)
P_MAIN = C * G_MAIN  # 114
P_REM = C * G_REM  # 76
SMOOTH = 1.0
IGNORE_INDEX = 255

_CACHE = {}


GROUP = 10  # tiles per PSUM S-group ([64, F], rows 6k..6k+6)
CONST_COLS = 1984


def _host_consts():
    """Zero-padded blockdiag matmul weights + per-row class vector.

    All matmul operands must sit at partition base 0, so row placement within
    the shared [64, F] S-tile (and row selection for the broadcast) is encoded
    in zero-padded weight variants:
      blkA_k  [114, 64]: col m=6k+g one-hot per row (c*6+g) -> S rows 6k..6k+6
      blkAr   [76, 64]:  col m=60+g                          -> S rows 60..64
      blkB_k  [64, 114]: row m=6k+g one-hot per col (c*6+g)  -> broadcast back
      blkBr   [64, 76]:  row m=60+g
    """
    bf16 = ml_dtypes.bfloat16
    cb = np.zeros((128, CONST_COLS), dtype=bf16)
    for k in range(GROUP):
        for c in range(C):
            for g in range(G_MAIN):
                cb[c * G_MAIN + g, k * 64 + 6 * k + g] = 1  # blkA_k
                cb[6 * k + g, 704 + k * P_MAIN + c * G_MAIN + g] = 1  # blkB_k
    for c in range(C):
        for g in range(G_REM):
            cb[c * G_REM + g, 640 + 60 + g] = 1  # blkAr
            cb[60 + g, 1844 + c * G_REM + g] = 1  # blkBr
    # Unused S rows would be 0 -> NaN reciprocal -> 0*NaN poisons the
    # broadcast matmul. Fillers give those rows a harmless positive S:
    cb[0, 60:64] = 1  # main groups (k=0 variant): rows 60:64 <- E row 0
    cb[:P_MAIN, 1920:1984] = cb[:P_MAIN, 0:64]  # blkA_last0: k=0 pattern...
    cb[0, 1920 + 60 : 1920 + 64] = 0  # ...without the 60:64 filler (rem lives there)
    cb[0, 1920 + 12 : 1920 + 60] = 1  # filler for rows 12:60 (last group)
    # cvec [128, 2] f32: col0 = class id of main row (i//6), col1 = rem (i//4)
    cv = np.zeros((128, 2), dtype=np.float32)
    cv[:P_MAIN, 0] = np.arange(P_MAIN) // G_MAIN
    cv[:P_REM, 1] = np.arange(P_REM) // G_REM
    return cb, cv


def _build_program():
    import concourse.bacc as bacc
    import concourse.mybir as mybir
    import concourse.tile as tile

    dt = mybir.dt
    AOP = mybir.AluOpType
    ACTF = mybir.ActivationFunctionType

    nc = bacc.Bacc("TRN2", target_bir_lowering=False, debug=False)
    logits_d = nc.declare_dram_parameter("logits", [C, HW], dt.float32, isOutput=False)
    targ_d = nc.declare_dram_parameter("targets", [HW], dt.uint8, isOutput=False)
    cb_d = nc.declare_dram_parameter(
        "consts_bf", [128, CONST_COLS], dt.bfloat16, isOutput=False
    )
    cv_d = nc.declare_dram_parameter("cvec", [128, 2], dt.float32, isOutput=False)
    out_d = nc.declare_dram_parameter("out", [4, 128], dt.float32, isOutput=True)

    with tile.TileContext(nc) as tc:
        with (
            tc.tile_pool(name="singles", bufs=1) as sing,
            tc.tile_pool(name="Lp", bufs=4) as Lp,
            tc.tile_pool(name="Ep", bufs=14) as Ep,
            tc.tile_pool(name="T8p", bufs=3) as T8p,
            tc.tile_pool(name="Tbp", bufs=3) as Tbp,
            tc.tile_pool(name="Rp", bufs=2) as Rp,
            tc.tile_pool(name="Rbsp", bufs=3) as Rbsp,
            tc.tile_pool(name="Wp", bufs=2) as Wp,
            tc.tile_pool(name="W2p", bufs=2) as W2p,
            tc.tile_pool(name="psS", bufs=2, space="PSUM") as psS,
            tc.tile_pool(name="psRb", bufs=2, space="PSUM") as psRb,
        ):
            consts = sing.tile([128, CONST_COLS], dt.bfloat16)
            nc.sync.dma_start(consts[:], cb_d[:])
            cvec = sing.tile([128, 2], dt.float32)
            nc.sync.dma_start(cvec[:], cv_d[:])
            blkA = [consts[0:P_MAIN, k * 64 : k * 64 + 64] for k in range(GROUP)]
            blkB = [
                consts[0:64, 704 + k * P_MAIN : 704 + (k + 1) * P_MAIN]
                for k in range(GROUP)
            ]
            blkAr = consts[0:P_REM, 640:704]
            blkBr = consts[0:64, 1844 : 1844 + P_REM]
            blkA_last0 = consts[0:P_MAIN, 1920:1984]

            # accum slots: one column per tile; col N_MAIN = remainder
            psSlots = sing.tile([P_MAIN, N_MAIN + 2], dt.float32)
            iSlots = sing.tile([P_MAIN, N_MAIN + 2], dt.float32)

            def value_passes(t, E, Rb_sb, T8, p_rows, g, cvcol):
                """TTR (PS partials) + targets broadcast + STT (I partials)."""
                Tb = Tbp.tile([p_rows, F], dt.uint8, tag="Tb")
                for c in range(C):
                    nc.sync.dma_start(Tb[c * g : (c + 1) * g, :], T8[:, :])
                from concourse.dve_ops import TENSOR_TENSOR_REDUCE

                Wt = Wp.tile([p_rows, F], dt.bfloat16, tag="W")
                # stock InstTensorTensorReduce fails on this HW path; the
                # custom-DVE clone (body=Src0*Src1*C1, accum=add seed C0) works
                nc.vector._custom_dve(
                    TENSOR_TENSOR_REDUCE,
                    out=Wt[:],
                    in0=E[:],
                    in1=Rb_sb[:],
                    s0=0.0,
                    s1=1.0,
                    accum_out=psSlots[0:p_rows, t : t + 1],
                )
                W2 = W2p.tile([p_rows, F], dt.bfloat16, tag="W2")
                nc.vector.scalar_tensor_tensor(
                    out=W2[:],
                    in0=Tb[:],
                    scalar=cvec[0:p_rows, cvcol : cvcol + 1],
                    in1=Wt[:],
                    op0=AOP.is_equal,
                    op1=AOP.mult,
                    accum_out=iSlots[0:p_rows, t : t + 1],
                )

            # 5 groups: 4x10 main tiles, then (2 main + remainder)
            groups = [list(range(10 * g0, 10 * g0 + 10)) for g0 in range(4)]
            groups.append([40, 41, "rem"])
            for gidx, members in enumerate(groups):
                SP = psS.tile([64, F], dt.float32, tag="S")
                E_tiles = []
                for k, t in enumerate(members):
                    if t == "rem":
                        L = Lp.tile([P_REM, F], dt.float32, tag="L")
                        nc.sync.dma_start(
                            L[:],
                            logits_d[:, REM_PX0:HW].rearrange(
                                "c (g f) -> c g f", g=G_REM
                            ),
                        )
                        E = Ep.tile([P_REM, F], dt.bfloat16, tag="E")
                        lhs = blkAr
                    else:
                        px0 = t * PXT
                        L = Lp.tile([P_MAIN, F], dt.float32, tag="L")
                        nc.sync.dma_start(
                            L[:],
                            logits_d[:, px0 : px0 + PXT].rearrange(
                                "c (g f) -> c g f", g=G_MAIN
                            ),
                        )
                        E = Ep.tile([P_MAIN, F], dt.bfloat16, tag="E")
                        lhs = blkA_last0 if (gidx == 4 and k == 0) else blkA[k]
                    nc.scalar.activation(E[:], L[:], ACTF.Exp)
                    E_tiles.append(E)
                    for j in range(2):
                        nc.tensor.matmul(
                            SP[:, j * 512 : (j + 1) * 512],
                            lhs,
                            E[:, j * 512 : (j + 1) * 512],
                            start=(k == 0),
                            stop=(k == len(members) - 1),
                        )
                R64f = Rp.tile([64, F], dt.float32, tag="Rf")
                nc.vector.reciprocal_approx_fast(R64f[:], SP[:])
                R64b = Rp.tile([64, F], dt.bfloat16, tag="Rb")
                nc.vector.tensor_copy(R64b[:], R64f[:])

                for k, t in enumerate(members):
                    if t == "rem":
                        p_rows, g, lhsb, cvcol, slot = P_REM, G_REM, blkBr, 1, N_MAIN
                        px0 = REM_PX0
                    else:
                        p_rows, g, lhsb, cvcol, slot = P_MAIN, G_MAIN, blkB[k], 0, t
                        px0 = t * PXT
                    Rb_ps = psRb.tile([p_rows, F], dt.float32, tag="Rbps")
                    for j in range(2):
                        nc.tensor.matmul(
                            Rb_ps[:, j * 512 : (j + 1) * 512],
                            lhsb,
                            R64b[:, j * 512 : (j + 1) * 512],
                            start=True,
                            stop=True,
                        )
                    Rb_sb = Rbsp.tile([p_rows, F], dt.bfloat16, tag="Rbsb")
                    nc.scalar.copy(Rb_sb[:], Rb_ps[:])
                    T8 = T8p.tile([g, F], dt.uint8, tag="T8")
                    nc.sync.dma_start(
                        T8[:],
                        targ_d[px0 : px0 + g * F].rearrange("(g f) -> g f", g=g),
                    )
                    value_passes(slot, E_tiles[k], Rb_sb, T8, p_rows, g, cvcol)

            # final: reduce main slot columns, DMA all partials out
            psA = sing.tile([P_MAIN, 1], dt.float32)
            iA = sing.tile([P_MAIN, 1], dt.float32)
            nc.vector.tensor_reduce(
                psA[:], psSlots[:, 0:N_MAIN], axis=mybir.AxisListType.X, op=AOP.add
            )
            nc.vector.tensor_reduce(
                iA[:], iSlots[:, 0:N_MAIN], axis=mybir.AxisListType.X, op=AOP.add
            )
            nc.sync.dma_start(out_d[0:1, 0:P_MAIN], psA[:])
            nc.sync.dma_start(out_d[1:2, 0:P_MAIN], iA[:])
            nc.sync.dma_start(out_d[2:3, 0:P_REM], psSlots[0:P_REM, N_MAIN : N_MAIN + 1])
            nc.sync.dma_start(out_d[3:4, 0:P_REM], iSlots[0:P_REM, N_MAIN : N_MAIN + 1])

    nc.compile()
    return nc


def _get_program():
    if "nc" not in _CACHE:
        _CACHE["nc"] = _build_program()
        _CACHE["consts"] = _host_consts()
    return _CACHE["nc"], _CACHE["consts"]


def _install_ntff_hook():
    """antenv.axon_hooks is missing in this image; synthesize it so
    run_bass_kernel_spmd(trace=True) can capture NTFF profiles via axon."""
    import types

    if "antenv.axon_hooks" in sys.modules:
        return
    mod = types.ModuleType("antenv.axon_hooks")
    _h = [None]
    mod.set_axon_ntff_profile_hook = lambda h: _h.__setitem__(0, h)
    mod.get_axon_ntff_profile_hook = lambda: _h[0]
    sys.modules["antenv.axon_hooks"] = mod
    import antenv

    antenv.axon_hooks = mod
    from trn_agent_boot.trn_boot import _ntff_profile_via_ctypes

    mod.set_axon_ntff_profile_hook(
        _ntff_profile_via_ctypes("/opt/axon/libaxon_pjrt.so")
    )


def _run_device(logits_np, targets_np, trace=False):
    """Run the SPMD kernel on 8 cores; returns (list of out arrays, results obj)."""
    from concourse.bass_utils import run_bass_kernel_spmd

    nc, (cb, cv) = _get_program()
    lg = np.asarray(logits_np, dtype=np.float32).reshape(B, C, HW)
    tg = np.asarray(targets_np).reshape(B, HW).astype(np.uint8)
    in_maps = [
        {"logits": lg[b], "targets": tg[b], "consts_bf": cb, "cvec": cv}
        for b in range(B)
    ]
    kwargs = {}
    if trace:
        _install_ntff_hook()
        kwargs = {"trace": True, "trace_cores": [0]}
    res = run_bass_kernel_spmd(nc, in_maps, core_ids=list(range(B)), **kwargs)
    outs = [res.results[b]["out"] for b in range(B)]
    return outs, res


def _combine(outs, targets_np):
    PS = np.zeros(C, dtype=np.float64)
    I = np.zeros(C, dtype=np.float64)
    for o in outs:
        PS += o[0, :P_MAIN].astype(np.float64).reshape(C, G_MAIN).sum(1)
        PS += o[2, :P_REM].astype(np.float64).reshape(C, G_REM).sum(1)
        I += o[1, :P_MAIN].astype(np.float64).reshape(C, G_MAIN).sum(1)
        I += o[3, :P_REM].astype(np.float64).reshape(C, G_REM).sum(1)
    t = np.asarray(targets_np).reshape(-1)
    valid = t != IGNORE_INDEX
    if not valid.any():
        return np.asarray(0.0, dtype=np.float32)
    CT = np.bincount(t[valid].astype(np.int64), minlength=C).astype(np.float64)
    dice = (2.0 * I + SMOOTH) / (PS + CT + SMOOTH)
    loss = (1.0 - dice).mean()
    return np.asarray(loss, dtype=np.float32)


def kernel(logits, targets):
    logits = np.asarray(logits)
    targets = np.asarray(targets)
    outs, _ = _run_device(logits, targets)
    return _combine(outs, targets)


# revision 21
# speedup vs baseline: 1.3560x; 1.3560x over previous
"""Dice loss kernel for Trainium2 (8 NeuronCores, SPMD data-parallel).

Problem: nn_DiceLoss — logits [8,19,512,512] f32, targets [8,512,512] int64.
  probs = softmax(logits, axis=1)
  PS[c] = sum_px probs[c,px]            (probs_sum)
  I[c]  = sum_px probs[t(px),px]*[t==c] (intersection)
  CT[c] = histogram(targets)            (counts; computed on host)
  dice  = (2I+1)/(PS+CT+1); loss = mean(1-dice)

Sharding: batch b -> core b. Each core reduces its [19, 512*512] slice to
per-class partials; host combines the 8 partial vectors and finishes.

Device layout (per core): classes-on-partitions, c-major rows (row = c*G+g):
  42 main tiles [114 = 19c x 6g, F=1024 pixels] + 1 remainder [76 = 19c x 4g, 1024].
  - ACT: E = exp(L) f32->bf16
  - PE:  S = blockdiag-ones.T @ E  -> per-pixel sum over the 19 classes,
         stacked 21 tiles deep in one PSUM tile [126, F]
  - DVE: r = approx 1/S (f32), cast bf16
  - PE:  broadcast r back to [114, F] (blockdiag.T @ r rows)
  - ACT: copy r-broadcast PSUM f32 -> SBUF bf16
  - DVE: tensor_tensor_reduce: W = E*r, accum_out = row sums -> PS partials
  - DMA: broadcast uint8 targets [6,F] -> [114,F] (19 small SBUF copies)
  - DVE: scalar_tensor_tensor: (T==c_row)*W, accum_out -> I partials
Outputs per core: out [4,128] f32 = PS/I partials for main + remainder rows.
"""

import functools
import sys

import numpy as np

sys.path.insert(0, "/opt/trn_rl_repo")

import ml_dtypes  # noqa: E402

B, C, H, W = 8, 19, 512, 512
HW = H * W  # 262144
F = 1024  # pixels per group column chunk
G_MAIN = 6  # groups per main tile -> 114 partitions
G_REM = 4  # groups in remainder tile -> 76 partitions
PXT = G_MAIN * F  # 6144 pixels per main tile
N_MAIN = 42  # main tiles (42*6144 = 258048)
REM_PX0 = N_MAIN * PXT  # 258048
BATCH = 21  # tiles per reciprocal batch ([126, F] PSUM stack)
P_MAIN = C * G_MAIN  # 114
P_REM = C * G_REM  # 76
SMOOTH = 1.0
IGNORE_INDEX = 255

_CACHE = {}


GROUP = 10  # tiles per PSUM S-group ([64, F], rows 6k..6k+6)
CONST_COLS = 1984


def _host_consts():
    """Zero-padded blockdiag matmul weights + per-row class vector.

    All matmul operands must sit at partition base 0, so row placement within
    the shared [64, F] S-tile (and row selection for the broadcast) is encoded
    in zero-padded weight variants:
      blkA_k  [114, 64]: col m=6k+g one-hot per row (c*6+g) -> S rows 6k..6k+6
      blkAr   [76, 64]:  col m=60+g                          -> S rows 60..64
      blkB_k  [64, 114]: row m=6k+g one-hot per col (c*6+g)  -> broadcast back
      blkBr   [64, 76]:  row m=60+g
    """
    bf16 = ml_dtypes.bfloat16
    cb = np.zeros((128, CONST_COLS), dtype=bf16)
    for k in range(GROUP):
        for c in range(C):
            for g in range(G_MAIN):
                cb[c * G_MAIN + g, k * 64 + 6 * k + g] = 1  # blkA_k
                cb[6 * k + g, 704 + k * P_MAIN + c * G_MAIN + g] = 1  # blkB_k
    for c in range(C):
        for g in range(G_REM):
            cb[c * G_REM + g, 640 + 60 + g] = 1  # blkAr
            cb[60 + g, 1844 + c * G_REM + g] = 1  # blkBr
    # Unused S rows would be 0 -> NaN reciprocal -> 0*NaN poisons the
    # broadcast matmul. Fillers give those rows a harmless positive S:
    cb[0, 60:64] = 1  # main groups (k=0 variant): rows 60:64 <- E row 0
    cb[:P_MAIN, 1920:1984] = cb[:P_MAIN, 0:64]  # blkA_last0: k=0 pattern...
    cb[0, 1920 + 60 : 1920 + 64] = 0  # ...without the 60:64 filler (rem lives there)
    cb[0, 1920 + 12 : 1920 + 60] = 1  # filler for rows 12:60 (last group)
    # cvec [128, 2] f32: col0 = class id of main row (i//6), col1 = rem (i//4)
    cv = np.zeros((128, 2), dtype=np.float32)
    cv[:P_MAIN, 0] = np.arange(P_MAIN) // G_MAIN
    cv[:P_REM, 1] = np.arange(P_REM) // G_REM
    return cb, cv


def _build_program():
    import concourse.bacc as bacc
    import concourse.mybir as mybir
    import concourse.tile as tile

    dt = mybir.dt
    AOP = mybir.AluOpType
    ACTF = mybir.ActivationFunctionType

    nc = bacc.Bacc("TRN2", target_bir_lowering=False, debug=False)
    logits_d = nc.declare_dram_parameter("logits", [C, HW], dt.float32, isOutput=False)
    targ_d = nc.declare_dram_parameter("targets", [HW], dt.uint8, isOutput=False)
    cb_d = nc.declare_dram_parameter(
        "consts_bf", [128, CONST_COLS], dt.bfloat16, isOutput=False
    )
    cv_d = nc.declare_dram_parameter("cvec", [128, 2], dt.float32, isOutput=False)
    out_d = nc.declare_dram_parameter("out", [4, 128], dt.float32, isOutput=True)

    with tile.TileContext(nc) as tc:
        with (
            tc.tile_pool(name="singles", bufs=1) as sing,
            tc.tile_pool(name="Lp", bufs=4) as Lp,
            tc.tile_pool(name="Ep", bufs=14) as Ep,
            tc.tile_pool(name="Tbp", bufs=3) as Tbp,
            tc.tile_pool(name="Rp", bufs=2) as Rp,
            tc.tile_pool(name="Rbsp", bufs=3) as Rbsp,
            tc.tile_pool(name="Wp", bufs=2) as Wp,
            tc.tile_pool(name="W2p", bufs=2) as W2p,
            tc.tile_pool(name="psS", bufs=2, space="PSUM") as psS,
            tc.tile_pool(name="psRb", bufs=2, space="PSUM") as psRb,
        ):
            consts = sing.tile([128, CONST_COLS], dt.bfloat16)
            nc.sync.dma_start(consts[:], cb_d[:])
            cvec = sing.tile([128, 2], dt.float32)
            nc.sync.dma_start(cvec[:], cv_d[:])
            blkA = [consts[0:P_MAIN, k * 64 : k * 64 + 64] for k in range(GROUP)]
            blkB = [
                consts[0:64, 704 + k * P_MAIN : 704 + (k + 1) * P_MAIN]
                for k in range(GROUP)
            ]
            blkAr = consts[0:P_REM, 640:704]
            blkBr = consts[0:64, 1844 : 1844 + P_REM]
            blkA_last0 = consts[0:P_MAIN, 1920:1984]

            # accum slots: one column per tile; col N_MAIN = remainder
            psSlots = sing.tile([P_MAIN, N_MAIN + 2], dt.float32)
            iSlots = sing.tile([P_MAIN, N_MAIN + 2], dt.float32)

            def value_passes(t, E, Rb_sb, px0, p_rows, g, cvcol):
                """TTR (PS partials) + targets broadcast + STT (I partials)."""
                Tb = Tbp.tile([p_rows, F], dt.uint8, tag="Tb")
                # one DMA per tile: u8 targets broadcast 19x via stride-0 dim,
                # issued on the scalar HWDGE to keep the sync sequencer free
                tsrc = (
                    targ_d[px0 : px0 + g * F]
                    .rearrange("(g f) -> g f", g=g)
                    .unsqueeze(0)
                    .broadcast_to([C, g, F])
                )
                nc.scalar.dma_start(Tb[:], tsrc)
                from concourse.dve_ops import TENSOR_TENSOR_REDUCE

                Wt = Wp.tile([p_rows, F], dt.bfloat16, tag="W")
                # stock InstTensorTensorReduce fails on this HW path; the
                # custom-DVE clone (body=Src0*Src1*C1, accum=add seed C0) works
                nc.vector._custom_dve(
                    TENSOR_TENSOR_REDUCE,
                    out=Wt[:],
                    in0=E[:],
                    in1=Rb_sb[:],
                    s0=0.0,
                    s1=1.0,
                    accum_out=psSlots[0:p_rows, t : t + 1],
                )
                W2 = W2p.tile([p_rows, F], dt.bfloat16, tag="W2")
                nc.vector.scalar_tensor_tensor(
                    out=W2[:],
                    in0=Tb[:],
                    scalar=cvec[0:p_rows, cvcol : cvcol + 1],
                    in1=Wt[:],
                    op0=AOP.is_equal,
                    op1=AOP.mult,
                    accum_out=iSlots[0:p_rows, t : t + 1],
                )

            # 5 groups: 4x10 main tiles, then (2 main + remainder)
            groups = [list(range(10 * g0, 10 * g0 + 10)) for g0 in range(4)]
            groups.append([40, 41, "rem"])
            for gidx, members in enumerate(groups):
                SP = psS.tile([64, F], dt.float32, tag="S")
                E_tiles = []
                for k, t in enumerate(members):
                    if t == "rem":
                        L = Lp.tile([P_REM, F], dt.float32, tag="L")
                        nc.sync.dma_start(
                            L[:],
                            logits_d[:, REM_PX0:HW].rearrange(
                                "c (g f) -> c g f", g=G_REM
                            ),
                        )
                        E = Ep.tile([P_REM, F], dt.bfloat16, tag="E")
                        lhs = blkAr
                    else:
                        px0 = t * PXT
                        L = Lp.tile([P_MAIN, F], dt.float32, tag="L")
                        nc.sync.dma_start(
                            L[:],
                            logits_d[:, px0 : px0 + PXT].rearrange(
                                "c (g f) -> c g f", g=G_MAIN
                            ),
                        )
                        E = Ep.tile([P_MAIN, F], dt.bfloat16, tag="E")
                        lhs = blkA_last0 if (gidx == 4 and k == 0) else blkA[k]
                    nc.scalar.activation(E[:], L[:], ACTF.Exp)
                    E_tiles.append(E)
                    for j in range(2):
                        nc.tensor.matmul(
                            SP[:, j * 512 : (j + 1) * 512],
                            lhs,
                            E[:, j * 512 : (j + 1) * 512],
                            start=(k == 0),
                            stop=(k == len(members) - 1),
                        )
                R64f = Rp.tile([64, F], dt.float32, tag="Rf")
                nc.vector.reciprocal_approx_fast(R64f[:], SP[:])
                R64b = Rp.tile([64, F], dt.bfloat16, tag="Rb")
                nc.vector.tensor_copy(R64b[:], R64f[:])

                for k, t in enumerate(members):
                    if t == "rem":
                        p_rows, g, lhsb, cvcol, slot = P_REM, G_REM, blkBr, 1, N_MAIN
                        px0 = REM_PX0
                    else:
                        p_rows, g, lhsb, cvcol, slot = P_MAIN, G_MAIN, blkB[k], 0, t
                        px0 = t * PXT
                    Rb_ps = psRb.tile([p_rows, F], dt.float32, tag="Rbps")
                    for j in range(2):
                        nc.tensor.matmul(
                            Rb_ps[:, j * 512 : (j + 1) * 512],
                            lhsb,
                            R64b[:, j * 512 : (j + 1) * 512],
                            start=True,
                            stop=True,
                        )
                    Rb_sb = Rbsp.tile([p_rows, F], dt.bfloat16, tag="Rbsb")
                    nc.scalar.copy(Rb_sb[:], Rb_ps[:])
                    value_passes(slot, E_tiles[k], Rb_sb, px0, p_rows, g, cvcol)

            # final: reduce main slot columns, DMA all partials out
            psA = sing.tile([P_MAIN, 1], dt.float32)
            iA = sing.tile([P_MAIN, 1], dt.float32)
            nc.vector.tensor_reduce(
                psA[:], psSlots[:, 0:N_MAIN], axis=mybir.AxisListType.X, op=AOP.add
            )
            nc.vector.tensor_reduce(
                iA[:], iSlots[:, 0:N_MAIN], axis=mybir.AxisListType.X, op=AOP.add
            )
            nc.sync.dma_start(out_d[0:1, 0:P_MAIN], psA[:])
            nc.sync.dma_start(out_d[1:2, 0:P_MAIN], iA[:])
            nc.sync.dma_start(out_d[2:3, 0:P_REM], psSlots[0:P_REM, N_MAIN : N_MAIN + 1])
            nc.sync.dma_start(out_d[3:4, 0:P_REM], iSlots[0:P_REM, N_MAIN : N_MAIN + 1])

    nc.compile()
    return nc


def _get_program():
    if "nc" not in _CACHE:
        _CACHE["nc"] = _build_program()
        _CACHE["consts"] = _host_consts()
    return _CACHE["nc"], _CACHE["consts"]


def _install_ntff_hook():
    """antenv.axon_hooks is missing in this image; synthesize it so
    run_bass_kernel_spmd(trace=True) can capture NTFF profiles via axon."""
    import types

    if "antenv.axon_hooks" in sys.modules:
        return
    mod = types.ModuleType("antenv.axon_hooks")
    _h = [None]
    mod.set_axon_ntff_profile_hook = lambda h: _h.__setitem__(0, h)
    mod.get_axon_ntff_profile_hook = lambda: _h[0]
    sys.modules["antenv.axon_hooks"] = mod
    import antenv

    antenv.axon_hooks = mod
    from trn_agent_boot.trn_boot import _ntff_profile_via_ctypes

    mod.set_axon_ntff_profile_hook(
        _ntff_profile_via_ctypes("/opt/axon/libaxon_pjrt.so")
    )


def _run_device(logits_np, targets_np, trace=False):
    """Run the SPMD kernel on 8 cores; returns (list of out arrays, results obj)."""
    from concourse.bass_utils import run_bass_kernel_spmd

    nc, (cb, cv) = _get_program()
    lg = np.asarray(logits_np, dtype=np.float32).reshape(B, C, HW)
    tg = np.asarray(targets_np).reshape(B, HW).astype(np.uint8)
    in_maps = [
        {"logits": lg[b], "targets": tg[b], "consts_bf": cb, "cvec": cv}
        for b in range(B)
    ]
    kwargs = {}
    if trace:
        _install_ntff_hook()
        kwargs = {"trace": True, "trace_cores": [0]}
    res = run_bass_kernel_spmd(nc, in_maps, core_ids=list(range(B)), **kwargs)
    outs = [res.results[b]["out"] for b in range(B)]
    return outs, res


def _combine(outs, targets_np):
    PS = np.zeros(C, dtype=np.float64)
    I = np.zeros(C, dtype=np.float64)
    for o in outs:
        PS += o[0, :P_MAIN].astype(np.float64).reshape(C, G_MAIN).sum(1)
        PS += o[2, :P_REM].astype(np.float64).reshape(C, G_REM).sum(1)
        I += o[1, :P_MAIN].astype(np.float64).reshape(C, G_MAIN).sum(1)
        I += o[3, :P_REM].astype(np.float64).reshape(C, G_REM).sum(1)
    t = np.asarray(targets_np).reshape(-1)
    valid = t != IGNORE_INDEX
    if not valid.any():
        return np.asarray(0.0, dtype=np.float32)
    CT = np.bincount(t[valid].astype(np.int64), minlength=C).astype(np.float64)
    dice = (2.0 * I + SMOOTH) / (PS + CT + SMOOTH)
    loss = (1.0 - dice).mean()
    return np.asarray(loss, dtype=np.float32)


def kernel(logits, targets):
    logits = np.asarray(logits)
    targets = np.asarray(targets)
    outs, _ = _run_device(logits, targets)
    return _combine(outs, targets)


# revision 26
# speedup vs baseline: 11.5164x; 8.4932x over previous
"""Dice loss kernel for Trainium2 (8 NeuronCores, SPMD data-parallel).

Problem: nn_DiceLoss — logits [8,19,512,512] f32, targets [8,512,512] int64.
  probs = softmax(logits, axis=1)
  PS[c] = sum_px probs[c,px]            (probs_sum)
  I[c]  = sum_px probs[t(px),px]*[t==c] (intersection)
  CT[c] = histogram(targets)            (counts; computed on host)
  dice  = (2I+1)/(PS+CT+1); loss = mean(1-dice)

Sharding: batch b -> core b. Each core reduces its [19, 512*512] slice to
per-class partials; host combines the 8 partial vectors and finishes.

Device layout (per core): logits viewed as [19*256 rows, 1024]; a tile is 128
consecutive rows = half of one class plane -> every DMA is a contiguous
512KB 2D load (spreads across all 16 SDMA engines; strided/3D dynamic DMAs
pin to engine 0 at ~26GB/s on this runtime — measured).

Per image-half h (128 pixel-blocks, partition-aligned across all tiles):
  - ACT: E_c = exp(L_c) f32->bf16                       (19 tiles)
  - PE:  S = sum_c E_c via identity-matmul PSUM accumulation
  - DVE: r = approx-recip(S) f32, cast bf16 (r is partition-aligned with E,
         so NO broadcast is needed anywhere)
  - DVE per class: M = (T==c) [tensor_scalar 4x], W = E*r [TT 2x],
         OW = M*W [TT 2x]
  - PE:  PS[c] += colsum(W), I[c] += colsum(OW) via ones-column lhsT into
         packed [19, 1024] PSUM accumulators (one accumulation group each)
  - DVE: final [19,1024] -> [19,1] reduces; tiny DMA out.
Outputs per core: out [2,32] f32: row0 = PS[19], row1 = I[19].
"""

import functools
import sys

import numpy as np

sys.path.insert(0, "/opt/trn_rl_repo")

import ml_dtypes  # noqa: E402

B, C, H, W = 8, 19, 512, 512
HW = H * W  # 262144
F = 1024  # pixels per group column chunk
G_MAIN = 6  # groups per main tile -> 114 partitions
G_REM = 4  # groups in remainder tile -> 76 partitions
PXT = G_MAIN * F  # 6144 pixels per main tile
N_MAIN = 42  # main tiles (42*6144 = 258048)
REM_PX0 = N_MAIN * PXT  # 258048
BATCH = 21  # tiles per reciprocal batch ([126, F] PSUM stack)
P_MAIN = C * G_MAIN  # 114
P_REM = C * G_REM  # 76
SMOOTH = 1.0
IGNORE_INDEX = 255

_CACHE = {}


ROWS = C * (HW // F)  # 4864 rows of [row, 1024] view of logits
N_H = 2  # image halves (128 row-blocks each)
CONST_COLS = 128 + C * C  # identity + 19 ones-column variants


def _host_consts():
    """identity [128,128] + per-class ones-column lhsT variants [128,19]."""
    bf16 = ml_dtypes.bfloat16
    cb = np.zeros((128, CONST_COLS), dtype=bf16)
    cb[:, 0:128] = np.eye(128, dtype=bf16)
    for c in range(C):
        cb[:, 128 + C * c + c] = 1  # onescol_c: column c all-ones
    return (cb,)


def _build_program():
    import concourse.bacc as bacc
    import concourse.mybir as mybir
    import concourse.tile as tile

    dt = mybir.dt
    AOP = mybir.AluOpType
    ACTF = mybir.ActivationFunctionType

    nc = bacc.Bacc("TRN2", target_bir_lowering=False, debug=False)
    logits_d = nc.declare_dram_parameter("logits", [ROWS, F], dt.float32, isOutput=False)
    targ_d = nc.declare_dram_parameter("targets", [HW // F, F], dt.bfloat16, isOutput=False)
    cb_d = nc.declare_dram_parameter("consts_bf", [128, CONST_COLS], dt.bfloat16, isOutput=False)
    out_d = nc.declare_dram_parameter("out", [2, 32], dt.float32, isOutput=True)

    with tile.TileContext(nc) as tc:
        with (
            tc.tile_pool(name="singles", bufs=1) as sing,
            tc.tile_pool(name="Lp", bufs=4) as Lp,
            tc.tile_pool(name="Ep", bufs=22) as Ep,
            tc.tile_pool(name="Tp", bufs=2) as Tp,
            tc.tile_pool(name="Rp", bufs=2) as Rp,
            tc.tile_pool(name="Mp", bufs=2) as Mp,
            tc.tile_pool(name="Wp", bufs=2) as Wp,
            tc.tile_pool(name="OWp", bufs=2) as OWp,
            tc.tile_pool(name="psS", bufs=2, space="PSUM") as psS,
            tc.tile_pool(name="psAcc", bufs=1, space="PSUM") as psAcc,
        ):
            consts = sing.tile([128, CONST_COLS], dt.bfloat16)
            nc.sync.dma_start(consts[:], cb_d[:])
            ident = consts[0:128, 0:128]
            onescol = [consts[0:128, 128 + C * c : 128 + C * (c + 1)] for c in range(C)]

            psPS = psAcc.tile([C, F], dt.float32, tag="ps")
            psI = psAcc.tile([C, F], dt.float32, tag="i")

            for h in range(N_H):
                Tt = Tp.tile([128, F], dt.bfloat16, tag="T")
                nc.sync.dma_start(Tt[:], targ_d[128 * h : 128 * (h + 1), :])
                SP = psS.tile([128, F], dt.float32, tag="S")
                Es = []
                for c in range(C):
                    r0 = c * (HW // F) + 128 * h
                    L = Lp.tile([128, F], dt.float32, tag="L")
                    nc.sync.dma_start(L[:], logits_d[r0 : r0 + 128, :])
                    E = Ep.tile([128, F], dt.bfloat16, tag="E")
                    nc.scalar.activation(E[:], L[:], ACTF.Exp)
                    Es.append(E)
                    for j in range(2):
                        nc.tensor.matmul(
                            SP[:, j * 512 : (j + 1) * 512],
                            ident,
                            E[:, j * 512 : (j + 1) * 512],
                            start=(c == 0),
                            stop=(c == C - 1),
                        )
                Rf = Rp.tile([128, F], dt.float32, tag="Rf")
                nc.vector.reciprocal_approx_fast(Rf[:], SP[:])
                Rb = Rp.tile([128, F], dt.bfloat16, tag="Rb")
                nc.vector.tensor_copy(Rb[:], Rf[:])

                for c in range(C):
                    M = Mp.tile([128, F], dt.bfloat16, tag="M")
                    nc.vector.tensor_scalar(
                        out=M[:], in0=Tt[:], scalar1=float(c), scalar2=None,
                        op0=AOP.is_equal,
                    )
                    W = Wp.tile([128, F], dt.bfloat16, tag="W")
                    nc.vector.tensor_tensor(out=W[:], in0=Es[c][:], in1=Rb[:], op=AOP.mult)
                    OW = OWp.tile([128, F], dt.bfloat16, tag="OW")
                    nc.vector.tensor_tensor(out=OW[:], in0=M[:], in1=W[:], op=AOP.mult)
                    first = h == 0 and c == 0
                    last = h == N_H - 1 and c == C - 1
                    for j in range(2):
                        nc.tensor.matmul(
                            psPS[:, j * 512 : (j + 1) * 512],
                            onescol[c],
                            W[:, j * 512 : (j + 1) * 512],
                            start=first,
                            stop=last,
                        )
                        nc.tensor.matmul(
                            psI[:, j * 512 : (j + 1) * 512],
                            onescol[c],
                            OW[:, j * 512 : (j + 1) * 512],
                            start=first,
                            stop=last,
                        )

            psv = sing.tile([C, 1], dt.float32)
            iv = sing.tile([C, 1], dt.float32)
            nc.vector.tensor_reduce(
                psv[:], psPS[:], axis=mybir.AxisListType.X, op=AOP.add
            )
            nc.vector.tensor_reduce(
                iv[:], psI[:], axis=mybir.AxisListType.X, op=AOP.add
            )
            nc.sync.dma_start(out_d[0:1, 0:C], psv[:])
            nc.sync.dma_start(out_d[1:2, 0:C], iv[:])

    nc.compile()
    return nc


def _get_program():
    if "nc" not in _CACHE:
        _CACHE["nc"] = _build_program()
        _CACHE["consts"] = _host_consts()
    return _CACHE["nc"], _CACHE["consts"]


def _install_ntff_hook():
    """antenv.axon_hooks is missing in this image; synthesize it so
    run_bass_kernel_spmd(trace=True) can capture NTFF profiles via axon."""
    import types

    if "antenv.axon_hooks" in sys.modules:
        return
    mod = types.ModuleType("antenv.axon_hooks")
    _h = [None]
    mod.set_axon_ntff_profile_hook = lambda h: _h.__setitem__(0, h)
    mod.get_axon_ntff_profile_hook = lambda: _h[0]
    sys.modules["antenv.axon_hooks"] = mod
    import antenv

    antenv.axon_hooks = mod
    from trn_agent_boot.trn_boot import _ntff_profile_via_ctypes

    mod.set_axon_ntff_profile_hook(
        _ntff_profile_via_ctypes("/opt/axon/libaxon_pjrt.so")
    )


def _run_device(logits_np, targets_np, trace=False):
    """Run the SPMD kernel on 8 cores; returns (list of out arrays, results obj)."""
    from concourse.bass_utils import run_bass_kernel_spmd

    nc, (cb,) = _get_program()
    lg = np.asarray(logits_np, dtype=np.float32).reshape(B, ROWS, F)
    tg = (
        np.asarray(targets_np)
        .reshape(B, HW // F, F)
        .astype(np.float32)
        .astype(ml_dtypes.bfloat16)
    )
    in_maps = [
        {"logits": lg[b], "targets": tg[b], "consts_bf": cb} for b in range(B)
    ]
    kwargs = {}
    if trace:
        _install_ntff_hook()
        kwargs = {"trace": True, "trace_cores": [0]}
    res = run_bass_kernel_spmd(nc, in_maps, core_ids=list(range(B)), **kwargs)
    outs = [res.results[b]["out"] for b in range(B)]
    return outs, res


def _combine(outs, targets_np):
    PS = np.zeros(C, dtype=np.float64)
    I = np.zeros(C, dtype=np.float64)
    for o in outs:
        PS += o[0, :C].astype(np.float64)
        I += o[1, :C].astype(np.float64)
    t = np.asarray(targets_np).reshape(-1)
    valid = t != IGNORE_INDEX
    if not valid.any():
        return np.asarray(0.0, dtype=np.float32)
    CT = np.bincount(t[valid].astype(np.int64), minlength=C).astype(np.float64)
    dice = (2.0 * I + SMOOTH) / (PS + CT + SMOOTH)
    loss = (1.0 - dice).mean()
    return np.asarray(loss, dtype=np.float32)


def kernel(logits, targets):
    logits = np.asarray(logits)
    targets = np.asarray(targets)
    outs, _ = _run_device(logits, targets)
    return _combine(outs, targets)


# revision 27
# speedup vs baseline: 11.5506x; 1.0030x over previous
"""Dice loss kernel for Trainium2 (8 NeuronCores, SPMD data-parallel).

Problem: nn_DiceLoss — logits [8,19,512,512] f32, targets [8,512,512] int64.
  probs = softmax(logits, axis=1)
  PS[c] = sum_px probs[c,px]            (probs_sum)
  I[c]  = sum_px probs[t(px),px]*[t==c] (intersection)
  CT[c] = histogram(targets)            (counts; computed on host)
  dice  = (2I+1)/(PS+CT+1); loss = mean(1-dice)

Sharding: batch b -> core b. Each core reduces its [19, 512*512] slice to
per-class partials; host combines the 8 partial vectors and finishes.

Device layout (per core): logits viewed as [19*256 rows, 1024]; a tile is 128
consecutive rows = half of one class plane -> every DMA is a contiguous
512KB 2D load (spreads across all 16 SDMA engines; strided/3D dynamic DMAs
pin to engine 0 at ~26GB/s on this runtime — measured).

Per image-half h (128 pixel-blocks, partition-aligned across all tiles):
  - ACT: E_c = exp(L_c) f32->bf16                       (19 tiles)
  - PE:  S = sum_c E_c via identity-matmul PSUM accumulation
  - DVE: r = approx-recip(S) f32, cast bf16 (r is partition-aligned with E,
         so NO broadcast is needed anywhere)
  - DVE per class: M = (T==c) [tensor_scalar 4x], W = E*r [TT 2x],
         OW = M*W [TT 2x]
  - PE:  PS[c] += colsum(W), I[c] += colsum(OW) via ones-column lhsT into
         packed [19, 1024] PSUM accumulators (one accumulation group each)
  - DVE: final [19,1024] -> [19,1] reduces; tiny DMA out.
Outputs per core: out [2,32] f32: row0 = PS[19], row1 = I[19].
"""

import functools
import sys

import numpy as np

sys.path.insert(0, "/opt/trn_rl_repo")

import ml_dtypes  # noqa: E402

B, C, H, W = 8, 19, 512, 512
HW = H * W  # 262144
F = 512  # pixels per row-block
G_MAIN = 6  # groups per main tile -> 114 partitions
G_REM = 4  # groups in remainder tile -> 76 partitions
PXT = G_MAIN * F  # 6144 pixels per main tile
N_MAIN = 42  # main tiles (42*6144 = 258048)
REM_PX0 = N_MAIN * PXT  # 258048
BATCH = 21  # tiles per reciprocal batch ([126, F] PSUM stack)
P_MAIN = C * G_MAIN  # 114
P_REM = C * G_REM  # 76
SMOOTH = 1.0
IGNORE_INDEX = 255

_CACHE = {}


ROWS = C * (HW // F)  # 9728 rows of the [row, 512] view of logits
N_H = 4  # pixel windows (128 row-blocks each)
CONST_COLS = 128 + C * C  # identity + 19 ones-column variants


def _host_consts():
    """identity [128,128] + per-class ones-column lhsT variants [128,19]."""
    bf16 = ml_dtypes.bfloat16
    cb = np.zeros((128, CONST_COLS), dtype=bf16)
    cb[:, 0:128] = np.eye(128, dtype=bf16)
    for c in range(C):
        cb[:, 128 + C * c + c] = 1  # onescol_c: column c all-ones
    return (cb,)


def _build_program():
    import concourse.bacc as bacc
    import concourse.mybir as mybir
    import concourse.tile as tile

    dt = mybir.dt
    AOP = mybir.AluOpType
    ACTF = mybir.ActivationFunctionType

    nc = bacc.Bacc("TRN2", target_bir_lowering=False, debug=False)
    logits_d = nc.declare_dram_parameter("logits", [ROWS, F], dt.bfloat16, isOutput=False)
    targ_d = nc.declare_dram_parameter("targets", [HW // F, F], dt.bfloat16, isOutput=False)
    cb_d = nc.declare_dram_parameter("consts_bf", [128, CONST_COLS], dt.bfloat16, isOutput=False)
    out_d = nc.declare_dram_parameter("out", [2, 32], dt.float32, isOutput=True)

    with tile.TileContext(nc) as tc:
        with (
            tc.tile_pool(name="singles", bufs=1) as sing,
            tc.tile_pool(name="Lp", bufs=4) as Lp,
            tc.tile_pool(name="Ep", bufs=22) as Ep,
            tc.tile_pool(name="Tp", bufs=2) as Tp,
            tc.tile_pool(name="Rp", bufs=2) as Rp,
            tc.tile_pool(name="Mp", bufs=2) as Mp,
            tc.tile_pool(name="Wp", bufs=2) as Wp,
            tc.tile_pool(name="psS", bufs=2, space="PSUM") as psS,
            tc.tile_pool(name="psAcc", bufs=1, space="PSUM") as psAcc,
        ):
            consts = sing.tile([128, CONST_COLS], dt.bfloat16)
            nc.sync.dma_start(consts[:], cb_d[:])
            ident = consts[0:128, 0:128]
            onescol = [consts[0:128, 128 + C * c : 128 + C * (c + 1)] for c in range(C)]

            psAll = psAcc.tile([C, 2 * F], dt.float32, tag="acc")  # [:, :F]=PS, [:, F:]=I

            for h in range(N_H):
                Tt = Tp.tile([128, F], dt.bfloat16, tag="T")
                nc.sync.dma_start(Tt[:], targ_d[128 * h : 128 * (h + 1), :])
                SP = psS.tile([128, F], dt.float32, tag="S")
                Es = []
                for c in range(C):
                    r0 = c * (HW // F) + 128 * h
                    L = Lp.tile([128, F], dt.bfloat16, tag="L")
                    nc.sync.dma_start(L[:], logits_d[r0 : r0 + 128, :])
                    E = Ep.tile([128, F], dt.bfloat16, tag="E")
                    nc.scalar.activation(E[:], L[:], ACTF.Exp)
                    Es.append(E)
                    nc.tensor.matmul(
                        SP[:], ident, E[:], start=(c == 0), stop=(c == C - 1)
                    )
                Rf = Rp.tile([128, F], dt.float32, tag="Rf")
                nc.vector.reciprocal_approx_fast(Rf[:], SP[:])
                Rb = Rp.tile([128, F], dt.bfloat16, tag="Rb")
                nc.vector.tensor_copy(Rb[:], Rf[:])

                for c in range(C):
                    M = Mp.tile([128, F], dt.bfloat16, tag="M")
                    nc.vector.tensor_scalar(
                        out=M[:], in0=Tt[:], scalar1=float(c), scalar2=None,
                        op0=AOP.is_equal,
                    )
                    # W and OW side by side in one tile: cols [0:F]=W, [F:2F]=OW
                    WOW = Wp.tile([128, 2 * F], dt.bfloat16, tag="W")
                    nc.vector.tensor_tensor(
                        out=WOW[:, 0:F], in0=Es[c][:], in1=Rb[:], op=AOP.mult
                    )
                    nc.vector.tensor_tensor(
                        out=WOW[:, F : 2 * F], in0=M[:], in1=WOW[:, 0:F], op=AOP.mult
                    )
                    first = h == 0 and c == 0
                    last = h == N_H - 1 and c == C - 1
                    for j in range(2):
                        nc.tensor.matmul(
                            psAll[:, j * F : (j + 1) * F],
                            onescol[c],
                            WOW[:, j * F : (j + 1) * F],
                            start=first,
                            stop=last,
                        )

            psv = sing.tile([C, 1], dt.float32)
            iv = sing.tile([C, 1], dt.float32)
            nc.vector.tensor_reduce(
                psv[:], psAll[:, 0:F], axis=mybir.AxisListType.X, op=AOP.add
            )
            nc.vector.tensor_reduce(
                iv[:], psAll[:, F : 2 * F], axis=mybir.AxisListType.X, op=AOP.add
            )
            nc.sync.dma_start(out_d[0:1, 0:C], psv[:])
            nc.sync.dma_start(out_d[1:2, 0:C], iv[:])

    nc.compile()
    return nc


def _get_program():
    if "nc" not in _CACHE:
        _CACHE["nc"] = _build_program()
        _CACHE["consts"] = _host_consts()
    return _CACHE["nc"], _CACHE["consts"]


def _install_ntff_hook():
    """antenv.axon_hooks is missing in this image; synthesize it so
    run_bass_kernel_spmd(trace=True) can capture NTFF profiles via axon."""
    import types

    if "antenv.axon_hooks" in sys.modules:
        return
    mod = types.ModuleType("antenv.axon_hooks")
    _h = [None]
    mod.set_axon_ntff_profile_hook = lambda h: _h.__setitem__(0, h)
    mod.get_axon_ntff_profile_hook = lambda: _h[0]
    sys.modules["antenv.axon_hooks"] = mod
    import antenv

    antenv.axon_hooks = mod
    from trn_agent_boot.trn_boot import _ntff_profile_via_ctypes

    mod.set_axon_ntff_profile_hook(
        _ntff_profile_via_ctypes("/opt/axon/libaxon_pjrt.so")
    )


def _run_device(logits_np, targets_np, trace=False):
    """Run the SPMD kernel on 8 cores; returns (list of out arrays, results obj)."""
    from concourse.bass_utils import run_bass_kernel_spmd

    nc, (cb,) = _get_program()
    lg = (
        np.asarray(logits_np, dtype=np.float32)
        .reshape(B, ROWS, F)
        .astype(ml_dtypes.bfloat16)
    )
    tg = (
        np.asarray(targets_np)
        .reshape(B, HW // F, F)
        .astype(np.float32)
        .astype(ml_dtypes.bfloat16)
    )
    in_maps = [
        {"logits": lg[b], "targets": tg[b], "consts_bf": cb} for b in range(B)
    ]
    kwargs = {}
    if trace:
        _install_ntff_hook()
        kwargs = {"trace": True, "trace_cores": [0]}
    res = run_bass_kernel_spmd(nc, in_maps, core_ids=list(range(B)), **kwargs)
    outs = [res.results[b]["out"] for b in range(B)]
    return outs, res


def _combine(outs, targets_np):
    PS = np.zeros(C, dtype=np.float64)
    I = np.zeros(C, dtype=np.float64)
    for o in outs:
        PS += o[0, :C].astype(np.float64)
        I += o[1, :C].astype(np.float64)
    t = np.asarray(targets_np).reshape(-1)
    valid = t != IGNORE_INDEX
    if not valid.any():
        return np.asarray(0.0, dtype=np.float32)
    CT = np.bincount(t[valid].astype(np.int64), minlength=C).astype(np.float64)
    dice = (2.0 * I + SMOOTH) / (PS + CT + SMOOTH)
    loss = (1.0 - dice).mean()
    return np.asarray(loss, dtype=np.float32)


def kernel(logits, targets):
    logits = np.asarray(logits)
    targets = np.asarray(targets)
    outs, _ = _run_device(logits, targets)
    return _combine(outs, targets)
